# revision 1
# baseline (speedup 1.0000x reference)
# Trainium2 Bass kernel for nn_ConditionalVariationalModule_75299366633595.
#
# Reference computation (see problem spec): a conditional VAE scan over
# S=256 timesteps. Per step t (batch B=1024):
#   prior_out = MLP3([h_t, z], pW*)          -> pm, plv      (2*128)
#   post_out  = MLP3([h_t, z, h_t], qW*)     -> qm, qlv
#   z_t = qm + eps_t * exp(0.5*qlv)
# Outputs: z, pm, plv, qm, qlv each [B, S, 128] (returned as a tuple).
#
# Strategy (8 cores, data-parallel over batch, 128 samples/core):
# - Everything is kept feature-major on device ([feature, batch]) so matmul
#   outputs feed the next layer with no transposes. Host does all transposes.
# - Posterior is the sequential recurrence. Per step, only the z-dependent
#   part of layer 1 is on the critical path; the h-part and biases are
#   accumulated into PSUM early (identity-matmul bias trick).
# - z is never materialized on the critical path: the next step's layer-1
#   matmuls take qm (bf16) and prod = eps*exp(0.5qlv+0.5b) (bf16) as two
#   separate rhs operands (W.T@(qm+prod) == W.T@qm + W.T@prod).
# - The prior MLP never feeds the recurrence, so it runs as batched N=512
#   GEMMs over 4-step token blocks, reading a bf16 z history from DRAM.
# - Matmuls in bf16 (fp32 matmul is 4x slower on TRN2 PE), accumulation fp32.

import os
import numpy as np
import ml_dtypes

import concourse.bass as bass
import concourse.mybir as mybir
import concourse.tile as tile
from concourse import bacc
from concourse import bass_utils

AF = mybir.ActivationFunctionType
F32 = mybir.dt.float32
BF16 = mybir.dt.bfloat16

NCORES = 8
B_TOTAL = 1024
BC = B_TOTAL // NCORES  # 128 batch per core
S = 256
D = 256  # input dim
L = 128  # latent dim
H = 256  # hidden dim


# --------------------------------------------------------------------------
# Device program
# --------------------------------------------------------------------------

def build_nc(n_steps=S, interleave=True):
    """Build the per-core Bass program (SPMD across 8 cores)."""
    nc = bacc.Bacc("TRN2", target_bir_lowering=False, debug=False,
                   num_devices=NCORES)

    # ---- DRAM I/O ----
    hT = nc.dram_tensor("hT", [2, 128, n_steps * BC], BF16, kind="ExternalInput")
    epsT = nc.dram_tensor("epsT", [128, n_steps * BC], BF16, kind="ExternalInput")
    z0T = nc.dram_tensor("z0T", [128, BC], BF16, kind="ExternalInput")

    wspec = {
        "qW1h": [2, 128, 256], "qW1z": [128, 256],
        "qW2": [2, 128, 256], "qW3": [2, 128, 256],
        "pW1h": [2, 128, 256], "pW1z": [128, 256],
        "pW2": [2, 128, 256], "pW3": [2, 128, 256],
        "B1q": [128, 256], "B2q": [128, 256],
        "ident": [128, 128],
    }
    wdram = {k: nc.dram_tensor(k, shp, BF16, kind="ExternalInput")
             for k, shp in wspec.items()}
    bspec = {
        "qb3m": [128, 1], "qb3lv": [128, 1], "qb3lvh": [128, 1],
        "pb1c": [128, 2], "pb2c": [128, 2],
        "pb3m": [128, 1], "pb3lv": [128, 1],
    }
    bdram = {k: nc.dram_tensor(k, shp, F32, kind="ExternalInput")
             for k, shp in bspec.items()}

    outs = {k: nc.dram_tensor(k, [n_steps, 128, BC], F32, kind="ExternalOutput")
            for k in ("z_out", "qm_out", "qlv_out", "pm_out", "plv_out")}

    n_blocks = n_steps // 4
    assert n_steps % 4 == 0

    with tile.TileContext(nc) as tc:
        with (
            tc.tile_pool(name="const", bufs=1) as const,
            tc.tile_pool(name="dram", bufs=1, space="DRAM") as dpool,
            tc.tile_pool(name="hp", bufs=6) as hpool,
            tc.tile_pool(name="ep", bufs=3) as epool,
            tc.tile_pool(name="sp", bufs=3) as spool,
            tc.tile_pool(name="blk", bufs=3) as bpool,
            tc.tile_pool(name="p3", bufs=3) as p3pool,
            tc.tile_pool(name="ps", bufs=2, space="PSUM") as psp,
        ):
            # ---- constants into SBUF ----
            w = {}
            for k in ("qW1h", "qW2", "qW3", "pW1h", "pW2", "pW3"):
                t_ = const.tile([128, 2, 256], BF16, tag=k)
                nc.sync.dma_start(t_[:], wdram[k].ap().rearrange("k d h -> d k h"))
                w[k] = t_
            for k in ("qW1z", "pW1z", "B1q", "B2q"):
                t_ = const.tile([128, 256], BF16, tag=k)
                nc.sync.dma_start(t_[:], wdram[k].ap())
                w[k] = t_
            ident = const.tile([128, 128], BF16, tag="ident")
            nc.sync.dma_start(ident[:], wdram["ident"].ap())
            z0t = const.tile([128, BC], BF16, tag="z0T")
            nc.sync.dma_start(z0t[:], z0T.ap())
            bias = {}
            for k, shp in bspec.items():
                t_ = const.tile(shp, F32, tag=k)
                nc.sync.dma_start(t_[:], bdram[k].ap())
                bias[k] = t_

            # bf16 z history in DRAM: slot s holds z_{s-1} (slot 0 = z0)
            zhist = dpool.tile([128, (n_steps + 1) * BC], BF16)
            nc.sync.dma_start(zhist[:, 0:BC], z0T.ap())

            # ---- streaming input tiles (4 steps per group) ----
            htiles, etiles = {}, {}

            def load_group(g):
                if g < 0 or g * 4 >= n_steps or g in htiles:
                    return
                ht = hpool.tile([128, 2, 4 * BC], BF16, tag="h")
                nc.sync.dma_start(
                    ht[:], hT.ap()[:, :, g * 4 * BC:(g + 1) * 4 * BC]
                    .rearrange("k d f -> d k f"))
                et = epool.tile([128, 4 * BC], BF16, tag="e")
                nc.sync.dma_start(et[:], epsT.ap()[:, g * 4 * BC:(g + 1) * 4 * BC])
                htiles[g] = ht
                etiles[g] = et

            load_group(0)
            load_group(1)

            qm_prev = prod_prev = None
            cur_qmblk = cur_prodblk = None

            HC = [(0, slice(0, 128)), (1, slice(128, 256))]

            def emit_prior_block(j):
                """Prior MLP for tokens [4j, 4j+4) — N=512 batched GEMMs."""
                zt_ = p3pool.tile([128, 4 * BC], BF16, tag="zblk")
                nc.sync.dma_start(zt_[:], zhist[:, j * 4 * BC:(j + 1) * 4 * BC])
                ht2 = hpool.tile([128, 2, 4 * BC], BF16, tag="h")
                nc.sync.dma_start(
                    ht2[:], hT.ap()[:, :, j * 4 * BC:(j + 1) * 4 * BC]
                    .rearrange("k d f -> d k f"))

                ps1 = [psp.tile([128, 4 * BC], F32, tag="p3ps", bufs=3, name=f"ps1_{hc}") for hc in (0, 1)]
                for hc, hs in HC:
                    nc.tensor.matmul(ps1[hc][:], w["pW1h"][:, 0, hs], ht2[:, 0, :],
                                     start=True, stop=False)
                    nc.tensor.matmul(ps1[hc][:], w["pW1h"][:, 1, hs], ht2[:, 1, :],
                                     start=False, stop=False)
                    nc.tensor.matmul(ps1[hc][:], w["pW1z"][:, hs], zt_[:],
                                     start=False, stop=True)
                h1p = p3pool.tile([128, 2, 4 * BC], BF16, tag="h1p")
                nc.scalar.activation(h1p[:, 0, :], ps1[0][:], AF.Relu,
                                     bias=bias["pb1c"][:, 0:1])
                nc.vector.tensor_scalar(h1p[:, 1, :], ps1[1][:],
                                        bias["pb1c"][:, 1:2], 0.0,
                                        mybir.AluOpType.add, mybir.AluOpType.max)
                ps2 = [psp.tile([128, 4 * BC], F32, tag="p3ps", bufs=3, name=f"ps2_{hc}") for hc in (0, 1)]
                for hc, hs in HC:
                    for kc in (0, 1):
                        nc.tensor.matmul(ps2[hc][:], w["pW2"][:, kc, hs],
                                         h1p[:, kc, :],
                                         start=(kc == 0), stop=(kc == 1))
                h2p = p3pool.tile([128, 2, 4 * BC], BF16, tag="h2p")
                nc.scalar.activation(h2p[:, 0, :], ps2[0][:], AF.Relu,
                                     bias=bias["pb2c"][:, 0:1])
                nc.vector.tensor_scalar(h2p[:, 1, :], ps2[1][:],
                                        bias["pb2c"][:, 1:2], 0.0,
                                        mybir.AluOpType.add, mybir.AluOpType.max)
                ps3 = [psp.tile([128, 4 * BC], F32, tag="p3ps", bufs=3, name=f"ps3_{hc}") for hc in (0, 1)]
                for hc, hs in HC:
                    for kc in (0, 1):
                        nc.tensor.matmul(ps3[hc][:], w["pW3"][:, kc, hs],
                                         h2p[:, kc, :],
                                         start=(kc == 0), stop=(kc == 1))
                pm_sb = p3pool.tile([128, 4 * BC], F32, tag="pm_sb")
                nc.vector.tensor_scalar_add(pm_sb[:], ps3[0][:], bias["pb3m"][:, 0:1])
                plv_sb = p3pool.tile([128, 4 * BC], F32, tag="plv_sb")
                nc.vector.tensor_scalar_add(plv_sb[:], ps3[1][:], bias["pb3lv"][:, 0:1])
                nc.sync.dma_start(
                    outs["pm_out"].ap()[j * 4:(j + 1) * 4].rearrange("t l b -> l t b"),
                    pm_sb[:].rearrange("l (t b) -> l t b", b=BC))
                nc.sync.dma_start(
                    outs["plv_out"].ap()[j * 4:(j + 1) * 4].rearrange("t l b -> l t b"),
                    plv_sb[:].rearrange("l (t b) -> l t b", b=BC))

            # ================= the scan =================
            for t in range(n_steps):
                g, sl = t // 4, t % 4
                if sl == 0:
                    load_group(g + 2)
                ht, et = htiles[g], etiles[g]
                bsl = slice(sl * BC, (sl + 1) * BC)

                # ---- layer 1 (posterior): bias + h-part early, z-part last
                psum1 = psp.tile([128, 256], F32, tag="l1", bufs=1)
                nc.tensor.matmul(psum1[:], ident[:], w["B1q"][:],
                                 start=True, stop=False)
                for hc, hs in HC:
                    for kc in (0, 1):
                        nc.tensor.matmul(psum1[:, hs], w["qW1h"][:, kc, hs],
                                         ht[:, kc, bsl], start=False, stop=False)
                for hc, hs in HC:
                    last = hc == 1
                    if t == 0:
                        nc.tensor.matmul(psum1[:, hs], w["qW1z"][:, hs], z0t[:],
                                         start=False, stop=last)
                    else:
                        nc.tensor.matmul(psum1[:, hs], w["qW1z"][:, hs], qm_prev,
                                         start=False, stop=False)
                        nc.tensor.matmul(psum1[:, hs], w["qW1z"][:, hs], prod_prev,
                                         start=False, stop=last)
                h1 = spool.tile([128, 256], BF16, tag="h1")
                nc.scalar.activation(h1[:], psum1[:], AF.Relu)

                # ---- layer 2
                psum2 = psp.tile([128, 256], F32, tag="l2", bufs=1)
                nc.tensor.matmul(psum2[:], ident[:], w["B2q"][:],
                                 start=True, stop=False)
                for hc, hs in HC:
                    for kc in (0, 1):
                        nc.tensor.matmul(psum2[:, hs], w["qW2"][:, kc, hs],
                                         h1[:, kc * 128:(kc + 1) * 128],
                                         start=False, stop=(hc == 1 and kc == 1))
                h2 = spool.tile([128, 256], BF16, tag="h2")
                nc.scalar.activation(h2[:], psum2[:], AF.Relu)

                # ---- layer 3 -> [qm | qlv]
                psum3 = psp.tile([128, 256], F32, tag="l3", bufs=2)
                for hc, hs in HC:
                    for kc in (0, 1):
                        nc.tensor.matmul(psum3[:, hs], w["qW3"][:, kc, hs],
                                         h2[:, kc * 128:(kc + 1) * 128],
                                         start=(hc == 0 and kc == 0),
                                         stop=(hc == 1 and kc == 1))

                # ---- recurrence + outputs
                ehalf = spool.tile([128, 128], F32, tag="eh")
                nc.scalar.activation(ehalf[:], psum3[:, 128:256], AF.Exp,
                                     bias=bias["qb3lvh"][:, 0:1], scale=0.5)

                if t + 1 < n_steps:
                    if (t + 1) % 4 == 0 or cur_qmblk is None:
                        cur_qmblk = bpool.tile([128, 4, BC], BF16, tag="qmb")
                        cur_prodblk = bpool.tile([128, 4, BC], BF16, tag="prb")
                    ws = (t + 1) % 4
                    qm_prev = cur_qmblk[:, ws, :]
                    prod_prev = cur_prodblk[:, ws, :]
                    nc.vector.tensor_scalar_add(qm_prev, psum3[:, 0:128],
                                                bias["qb3m"][:, 0:1])
                    nc.vector.tensor_mul(prod_prev, ehalf[:], et[:, bsl])

                if sl == 0:
                    ob_qm = spool.tile([128, 4, BC], F32, tag="ob_qm")
                    ob_qlv = spool.tile([128, 4, BC], F32, tag="ob_qlv")
                    ob_z = spool.tile([128, 4, BC], F32, tag="ob_z")
                    ob_zbf = spool.tile([128, 4, BC], BF16, tag="ob_zbf")
                qm_f = ob_qm[:, sl, :]
                nc.vector.tensor_scalar_add(qm_f, psum3[:, 0:128],
                                            bias["qb3m"][:, 0:1])
                nc.vector.tensor_scalar_add(ob_qlv[:, sl, :], psum3[:, 128:256],
                                            bias["qb3lv"][:, 0:1])
                prod_f = spool.tile([128, 128], F32, tag="prodf")
                nc.gpsimd.tensor_mul(prod_f[:], ehalf[:], et[:, bsl])
                nc.gpsimd.tensor_add(ob_z[:, sl, :], qm_f, prod_f[:])
                nc.gpsimd.tensor_copy(ob_zbf[:, sl, :], ob_z[:, sl, :])
                if sl == 3:
                    g4 = slice(g * 4, g * 4 + 4)
                    nc.sync.dma_start(
                        outs["z_out"].ap()[g4].rearrange("t l b -> l t b"), ob_z[:])
                    nc.sync.dma_start(
                        outs["qm_out"].ap()[g4].rearrange("t l b -> l t b"), ob_qm[:])
                    nc.sync.dma_start(
                        outs["qlv_out"].ap()[g4].rearrange("t l b -> l t b"), ob_qlv[:])
                    nc.sync.dma_start(
                        zhist[:, (g * 4 + 1) * BC:(g * 4 + 5) * BC], ob_zbf[:])

                # interleave prior blocks into the scan's engine gaps,
                # lagged 2 blocks so the z-history DMA round-trip never
                # head-of-line-blocks the PE queue
                if interleave and t % 4 == 3 and t >= 11:
                    emit_prior_block((t - 3) // 4 - 2)

            if interleave:
                emit_prior_block(n_blocks - 2)
                emit_prior_block(n_blocks - 1)
            else:
                for j in range(n_blocks):
                    emit_prior_block(j)

    nc.compile()
    return nc


# --------------------------------------------------------------------------
# Host-side data prep
# --------------------------------------------------------------------------

def prep_inputs(encoder_features, prev_latent, eps,
                pW1, pb1, pW2, pb2, pW3, pb3,
                qW1, qb1, qW2, qb2, qW3, qb3, n_steps=S):
    bf = ml_dtypes.bfloat16
    f32 = np.float32
    nco = NCORES

    enc = np.asarray(encoder_features, f32)[:, :n_steps]
    epsv = np.asarray(eps, f32)[:, :n_steps]
    prev = np.asarray(prev_latent, f32)

    # [core, kc, d, s, b]
    hT = np.ascontiguousarray(
        enc.reshape(nco, BC, n_steps, 2, 128).transpose(0, 3, 4, 2, 1)
    ).reshape(nco, 2, 128, n_steps * BC).astype(bf)
    epsT = np.ascontiguousarray(
        epsv.reshape(nco, BC, n_steps, 128).transpose(0, 3, 2, 1)
    ).reshape(nco, 128, n_steps * BC).astype(bf)
    z0T = np.ascontiguousarray(
        prev.reshape(nco, BC, 128).transpose(0, 2, 1)).astype(bf)

    def wchunks(wmat):  # [256, H'] -> [2, 128, H']
        return np.ascontiguousarray(np.asarray(wmat, f32).reshape(2, 128, -1)).astype(bf)

    qW1 = np.asarray(qW1, f32)
    pW1 = np.asarray(pW1, f32)
    shared = {
        "qW1h": wchunks(qW1[0:256] + qW1[384:640]),
        "qW1z": np.ascontiguousarray(qW1[256:384]).astype(bf),
        "qW2": wchunks(qW2), "qW3": wchunks(qW3),
        "pW1h": wchunks(pW1[0:256]),
        "pW1z": np.ascontiguousarray(pW1[256:384]).astype(bf),
        "pW2": wchunks(pW2), "pW3": wchunks(pW3),
        "B1q": np.ascontiguousarray(np.broadcast_to(
            np.asarray(qb1, f32).reshape(2, 128).T[:, :, None],
            (128, 2, BC))).reshape(128, 256).astype(bf),
        "B2q": np.ascontiguousarray(np.broadcast_to(
            np.asarray(qb2, f32).reshape(2, 128).T[:, :, None],
            (128, 2, BC))).reshape(128, 256).astype(bf),
        "ident": np.eye(128, dtype=f32).astype(bf),
        "qb3m": np.asarray(qb3, f32)[0:128].reshape(128, 1).copy(),
        "qb3lv": np.asarray(qb3, f32)[128:256].reshape(128, 1).copy(),
        "qb3lvh": (0.5 * np.asarray(qb3, f32)[128:256]).reshape(128, 1).copy(),
        "pb1c": np.ascontiguousarray(np.asarray(pb1, f32).reshape(2, 128).T),
        "pb2c": np.ascontiguousarray(np.asarray(pb2, f32).reshape(2, 128).T),
        "pb3m": np.asarray(pb3, f32)[0:128].reshape(128, 1).copy(),
        "pb3lv": np.asarray(pb3, f32)[128:256].reshape(128, 1).copy(),
    }
    in_maps = []
    for c in range(nco):
        m = {"hT": hT[c], "epsT": epsT[c], "z0T": z0T[c]}
        m.update(shared)
        in_maps.append(m)
    return in_maps


def unshard(results, n_steps=S):
    """results: list of per-core dicts with [n_steps, 128(l), BC(b)] f32."""
    def full(name):
        # -> [B_TOTAL, n_steps, 128]
        per = [r[name].transpose(2, 0, 1) for r in results]
        return np.ascontiguousarray(np.concatenate(per, axis=0))
    return (full("z_out"), full("pm_out"), full("plv_out"),
            full("qm_out"), full("qlv_out"))


_NC_CACHE = {}


def get_nc(n_steps=S, interleave=True):
    key = (n_steps, interleave)
    if key not in _NC_CACHE:
        _NC_CACHE[key] = build_nc(n_steps, interleave)
    return _NC_CACHE[key]


def kernel(**inputs):
    in_maps = prep_inputs(**inputs)
    nc = get_nc(S)
    res = bass_utils.run_bass_kernel_spmd(
        nc, in_maps, core_ids=list(range(NCORES)), trace=False)
    return unshard(res.results)



# revision 9
# speedup vs baseline: 1.2481x; 1.2481x over previous
# Trainium2 Bass kernel for nn_ConditionalVariationalModule_75299366633595.
#
# Reference computation (see problem spec): a conditional VAE scan over
# S=256 timesteps. Per step t (batch B=1024):
#   prior_out = MLP3([h_t, z], pW*)          -> pm, plv      (2*128)
#   post_out  = MLP3([h_t, z, h_t], qW*)     -> qm, qlv
#   z_t = qm + eps_t * exp(0.5*qlv)
# Outputs: z, pm, plv, qm, qlv each [B, S, 128] (returned as a tuple).
#
# Strategy (8 cores, data-parallel over batch, 128 samples/core):
# - Feature-major on device ([feature, batch]); host does all transposes.
# - The posterior recurrence is latency-bound: per step the chain is
#   zmm -> relu1 -> L2 -> relu2 -> L3(lv) -> exp -> prod -> zmm'.
#   Chain ops are placed on the lowest-latency engines (DVE relus,
#   ACT exp, DVE 2x-bf16 multiply), the qlv half of L3 is computed
#   first so exp starts early, and everything else (prior MLP, h-part
#   matmuls, bias-ident matmuls, output staging) fills the PE bubbles.
# - z is never materialized on the critical path: layer-1 takes qm and
#   prod = eps*exp(0.5qlv+0.5b) as two rhs operands.
# - The prior MLP runs as batched N=512 GEMMs over 4-step token blocks,
#   interleaved at sub-step granularity, reading z and h from SBUF.
# - All outputs staged/DMA'd in bf16 (host upcasts); matmuls bf16.

import os
import numpy as np
import ml_dtypes

import concourse.bass as bass
import concourse.mybir as mybir
import concourse.tile as tile
from concourse import bacc
from concourse import bass_utils

AF = mybir.ActivationFunctionType
ALU = mybir.AluOpType
F32 = mybir.dt.float32
BF16 = mybir.dt.bfloat16

NCORES = 8
B_TOTAL = 1024
BC = B_TOTAL // NCORES  # 128 batch per core
S = 256
D = 256  # input dim
L = 128  # latent dim
H = 256  # hidden dim


# --------------------------------------------------------------------------
# Device program
# --------------------------------------------------------------------------

def build_nc(n_steps=S):
    """Build the per-core Bass program (SPMD across 8 cores)."""
    nc = bacc.Bacc("TRN2", target_bir_lowering=False, debug=False,
                   num_devices=NCORES)

    n_blocks = n_steps // 4
    assert n_steps % 4 == 0

    # ---- DRAM I/O ----
    hT = nc.dram_tensor("hT", [2, 128, n_steps * BC], BF16, kind="ExternalInput")
    epsT = nc.dram_tensor("epsT", [128, n_steps * BC], BF16, kind="ExternalInput")
    z0T = nc.dram_tensor("z0T", [128, BC], BF16, kind="ExternalInput")

    wspec = {
        "qW1h": [2, 128, 256], "qW1z": [128, 256],
        "qW2": [2, 128, 256], "qW3": [2, 128, 256],
        "pW1h": [2, 128, 256], "pW1z": [128, 256],
        "pW2": [2, 128, 256], "pW3": [2, 128, 256],
        "B1q": [128, 256], "B2q": [128, 256],
        "ident": [128, 128],
    }
    wdram = {k: nc.dram_tensor(k, shp, BF16, kind="ExternalInput")
             for k, shp in wspec.items()}
    bspec = {
        "qb3m": [128, 1], "qb3lv": [128, 1], "qb3lvh": [128, 1],
        "pb1c": [128, 2], "pb2c": [128, 2],
        "pb3m": [128, 1], "pb3lv": [128, 1],
    }
    bdram = {k: nc.dram_tensor(k, shp, F32, kind="ExternalInput")
             for k, shp in bspec.items()}

    # outputs, all bf16 (host upcasts); qm is slot-shifted by one step
    outs = {
        "z_out": nc.dram_tensor("z_out", [n_blocks, 128, 4, BC], BF16,
                                kind="ExternalOutput"),
        "qm_out": nc.dram_tensor("qm_out", [n_blocks + 1, 128, 4, BC], BF16,
                                 kind="ExternalOutput"),
        "qlv_out": nc.dram_tensor("qlv_out", [n_blocks, 128, 4, BC], BF16,
                                  kind="ExternalOutput"),
        "pm_out": nc.dram_tensor("pm_out", [n_blocks, 128, 4, BC], BF16,
                                 kind="ExternalOutput"),
        "plv_out": nc.dram_tensor("plv_out", [n_blocks, 128, 4, BC], BF16,
                                  kind="ExternalOutput"),
    }

    with tile.TileContext(nc) as tc:
        with (
            tc.tile_pool(name="const", bufs=1) as const,
            tc.tile_pool(name="hp", bufs=7) as hpool,
            tc.tile_pool(name="ep", bufs=3) as epool,
            tc.tile_pool(name="sp", bufs=3) as spool,
            tc.tile_pool(name="zb", bufs=6) as zpool,
            tc.tile_pool(name="blk", bufs=3) as bpool,
            tc.tile_pool(name="p3", bufs=3) as p3pool,
            tc.tile_pool(name="ps", bufs=2, space="PSUM") as psp,
        ):
            # ---- constants into SBUF; scan-critical weights first, and
            # spread across three DMA queues so loads overlap ----
            w = {}
            bias = {}

            def wload(k, eng):
                t_ = const.tile([128, 2, 256] if len(wspec[k]) == 3
                                else list(wspec[k]), BF16, tag=k)
                if len(wspec[k]) == 3:
                    eng.dma_start(t_[:], wdram[k].ap().rearrange("k d h -> d k h"))
                else:
                    eng.dma_start(t_[:], wdram[k].ap())
                w[k] = t_

            def bload(k, eng):
                t_ = const.tile(list(bspec[k]), F32, tag=k)
                eng.dma_start(t_[:], bdram[k].ap())
                bias[k] = t_

            # scan-critical constants first: ident/B1q on ACT (first matmul
            # needs them), z0/h/qW1 on SP, later-layer weights behind them
            for k in ("ident", "B1q"):
                wload(k, nc.scalar)
            z0t = const.tile([128, BC], BF16, tag="z0T")
            nc.sync.dma_start(z0t[:], z0T.ap())
            ident = w["ident"]

            # ---- streaming input tiles (4 steps per group) ----
            htiles, etiles = {}, {}

            def load_group(g):
                if g < 0 or g * 4 >= n_steps or g in htiles:
                    return
                ht = hpool.tile([128, 2, 4 * BC], BF16, tag="h")
                nc.sync.dma_start(
                    ht[:], hT.ap()[:, :, g * 4 * BC:(g + 1) * 4 * BC]
                    .rearrange("k d f -> d k f"))
                et = epool.tile([128, 4 * BC], BF16, tag="e")
                nc.sync.dma_start(et[:], epsT.ap()[:, g * 4 * BC:(g + 1) * 4 * BC])
                htiles[g] = ht
                etiles[g] = et

            load_group(0)
            for k in ("qW1h", "qW1z"):
                wload(k, nc.sync)
            for k in ("qW2", "B2q", "qW3"):
                wload(k, nc.scalar)
            for k in ("qb3m", "qb3lv", "qb3lvh"):
                bload(k, nc.scalar)
            load_group(1)
            load_group(2)

            # prior-side constants: on SP after the first input groups
            # (not needed until the first prior block drains, ~step 8)
            for k in ("pW1h", "pW1z", "pW2", "pW3"):
                wload(k, nc.sync)
            for k in ("pb1c", "pb2c", "pb3m", "pb3lv"):
                bload(k, nc.sync)

            zbftiles = {}   # block g -> ob_zbf tile (z_t bf16, slots 0..3)

            HC = [(0, slice(0, 128)), (1, slice(128, 256))]

            # ---------- prior MLP work queue (sub-step interleave) ----------
            from collections import deque
            pwork = deque()

            def drain(n):
                for _ in range(n):
                    if pwork:
                        pwork.popleft()()

            def enqueue_prior_block(j):
                """Prior MLP for tokens [4j, 4j+4) as a list of small work
                units; z and h come from SBUF (no DRAM round-trip)."""
                if j < 0 or j >= n_blocks:
                    return
                ht2 = htiles[j]
                zprev = z0t[:] if j == 0 else zbftiles[j - 1][:, 3, :]
                zcur = zbftiles[j]
                st = {}

                def mk_ps1(hc, hs):
                    def f():
                        ps = psp.tile([128, 4 * BC], F32, tag="pps",
                                      bufs=3, name=f"pps1_{j}_{hc}")
                        st[("ps1", hc)] = ps
                        nc.tensor.matmul(ps[:], w["pW1h"][:, 0, hs],
                                         ht2[:, 0, :], start=True, stop=False)
                        nc.tensor.matmul(ps[:], w["pW1h"][:, 1, hs],
                                         ht2[:, 1, :], start=False, stop=False)
                    return f

                def mk_ps1z(hc, hs):
                    def f():
                        ps = st[("ps1", hc)]
                        nc.tensor.matmul(ps[:, 0:BC], w["pW1z"][:, hs],
                                         zprev, start=False, stop=False)
                        nc.tensor.matmul(ps[:, BC:4 * BC], w["pW1z"][:, hs],
                                         zcur[:, 0:3, :], start=False, stop=True)
                    return f

                def mk_relu(layer, hc, bkey, nxt):
                    def f():
                        if layer not in st:
                            st[layer] = p3pool.tile([128, 2, 4 * BC], BF16,
                                                    tag=f"h{layer}",
                                                    name=f"ph{layer}_{j}")
                        nc.scalar.activation(st[layer][:, hc, :],
                                             st[(f"ps{layer}", hc)][:],
                                             AF.Relu, bias=bias[bkey][:, hc:hc + 1])
                    return f

                def mk_ps(layer, hc, hs, wkey, prev):
                    def f():
                        ps = psp.tile([128, 4 * BC], F32, tag="pps",
                                      bufs=3, name=f"pps{layer}_{j}_{hc}")
                        st[(f"ps{layer}", hc)] = ps
                        for kc in (0, 1):
                            nc.tensor.matmul(ps[:], w[wkey][:, kc, hs],
                                             st[prev][:, kc, :],
                                             start=(kc == 0), stop=(kc == 1))
                    return f

                def mk_out(hc, bkey, oname):
                    def f():
                        ot = p3pool.tile([128, 4 * BC], BF16, tag=f"o{oname}")
                        nc.scalar.activation(ot[:], st[("ps3", hc)][:],
                                             AF.Identity,
                                             bias=bias[bkey][:, 0:1])
                        nc.sync.dma_start(
                            outs[oname].ap()[j].rearrange("l t b -> l (t b)"),
                            ot[:])
                    return f

                for hc, hs in HC:
                    pwork.append(mk_ps1(hc, hs))
                    pwork.append(mk_ps1z(hc, hs))
                pwork.append(mk_relu(1, 0, "pb1c", None))
                pwork.append(mk_relu(1, 1, "pb1c", None))
                for hc, hs in HC:
                    pwork.append(mk_ps(2, hc, hs, "pW2", 1))
                pwork.append(mk_relu(2, 0, "pb2c", None))
                pwork.append(mk_relu(2, 1, "pb2c", None))
                for hc, hs in HC:
                    pwork.append(mk_ps(3, hc, hs, "pW3", 2))
                pwork.append(mk_out(0, "pb3m", "pm_out"))
                pwork.append(mk_out(1, "pb3lv", "plv_out"))

            # ================= the scan =================
            # Software-pipelined emission: layer-1 bias/h matmuls for step
            # t+1 are emitted inside step t (they fill the tail stall while
            # exp/prod compute), so the loop body starts with the z matmuls.
            qm_prev = prod_prev = None
            cur_qmblk = cur_prodblk = None
            ob_zbf = ob_qlv = None

            def emit_l1_base(t):
                """ident-bias + h-part of posterior layer 1 for step t."""
                g, sl = t // 4, t % 4
                ht = htiles[g]
                bsl = slice(sl * BC, (sl + 1) * BC)
                ps = psp.tile([128, 256], F32, tag="l1", bufs=2,
                              name=f"psum1_{t}")
                nc.tensor.matmul(ps[:], ident[:], w["B1q"][:],
                                 start=True, stop=False)
                for hc, hs in HC:
                    for kc in (0, 1):
                        nc.tensor.matmul(ps[:, hs], w["qW1h"][:, kc, hs],
                                         ht[:, kc, bsl], start=False, stop=False)
                return ps

            psum1 = emit_l1_base(0)

            for t in range(n_steps):
                g, sl = t // 4, t % 4
                if sl == 0:
                    load_group(g + 3)
                    enqueue_prior_block(g - 2)
                et = etiles[g]
                bsl = slice(sl * BC, (sl + 1) * BC)

                # ---- layer 1 z-part (the chain input)
                for hc, hs in HC:
                    last = hc == 1
                    if t == 0:
                        nc.tensor.matmul(psum1[:, hs], w["qW1z"][:, hs], z0t[:],
                                         start=False, stop=last)
                    else:
                        nc.tensor.matmul(psum1[:, hs], w["qW1z"][:, hs], qm_prev,
                                         start=False, stop=False)
                        nc.tensor.matmul(psum1[:, hs], w["qW1z"][:, hs], prod_prev,
                                         start=False, stop=last)
                h1 = spool.tile([128, 256], BF16, tag="h1")
                nc.vector.tensor_scalar_max(h1[:], psum1[:], 0.0)

                # window (a): PE waits for relu1 — fill with bias mm + prior
                psum2 = psp.tile([128, 256], F32, tag="l2", bufs=1)
                nc.tensor.matmul(psum2[:], ident[:], w["B2q"][:],
                                 start=True, stop=False)
                drain(1)

                # ---- layer 2
                for hc, hs in HC:
                    for kc in (0, 1):
                        nc.tensor.matmul(psum2[:, hs], w["qW2"][:, kc, hs],
                                         h1[:, kc * 128:(kc + 1) * 128],
                                         start=False, stop=(hc == 1 and kc == 1))
                h2 = spool.tile([128, 256], BF16, tag="h2")
                nc.vector.tensor_scalar_max(h2[:], psum2[:], 0.0)

                # window (b): PE waits for relu2 — fill with prior
                drain(1)

                # ---- layer 3: qlv half FIRST (exp is the critical tail)
                psum3lv = psp.tile([128, 128], F32, tag="l3lv", bufs=1)
                for kc in (0, 1):
                    nc.tensor.matmul(psum3lv[:], w["qW3"][:, kc, 128:256],
                                     h2[:, kc * 128:(kc + 1) * 128],
                                     start=(kc == 0), stop=(kc == 1))
                psum3qm = psp.tile([128, 128], F32, tag="l3qm", bufs=1)
                for kc in (0, 1):
                    nc.tensor.matmul(psum3qm[:], w["qW3"][:, kc, 0:128],
                                     h2[:, kc * 128:(kc + 1) * 128],
                                     start=(kc == 0), stop=(kc == 1))

                # ---- recurrence tail (ACT/DVE) + window (c) PE fill
                ehalf = spool.tile([128, 128], BF16, tag="eh")
                nc.scalar.activation(ehalf[:], psum3lv[:], AF.Exp,
                                     bias=bias["qb3lvh"][:, 0:1], scale=0.5)

                ws = (t + 1) % 4
                if t == 0 or ws == 0:
                    cur_qmblk = bpool.tile([128, 4, BC], BF16, tag="qmb")
                    cur_prodblk = bpool.tile([128, 4, BC], BF16, tag="prb")
                qm_prev = cur_qmblk[:, ws, :]
                prod_prev = cur_prodblk[:, ws, :]
                nc.vector.tensor_scalar_add(qm_prev, psum3qm[:],
                                            bias["qb3m"][:, 0:1])
                nc.vector.tensor_tensor(prod_prev, ehalf[:], et[:, bsl],
                                        ALU.mult)

                # window (c): PE waits for exp/prod — fill with next L1
                if t + 1 < n_steps:
                    psum1 = emit_l1_base(t + 1)

                # ---- output staging (bf16)
                if sl == 0:
                    ob_zbf = zpool.tile([128, 4, BC], BF16, tag="zb")
                    zbftiles[g] = ob_zbf
                    ob_qlv = spool.tile([128, 4, BC], BF16, tag="qlvb", bufs=2)
                nc.gpsimd.tensor_add(ob_zbf[:, sl, :], qm_prev, prod_prev)
                nc.scalar.activation(ob_qlv[:, sl, :], psum3lv[:], AF.Identity,
                                     bias=bias["qb3lv"][:, 0:1])

                if ws == 3 or t == n_steps - 1:
                    nc.sync.dma_start(
                        outs["qm_out"].ap()[(t + 1) // 4]
                        .rearrange("l t b -> l (t b)"),
                        cur_qmblk[:].rearrange("l t b -> l (t b)"))
                if sl == 3:
                    nc.sync.dma_start(
                        outs["z_out"].ap()[g].rearrange("l t b -> l (t b)"),
                        ob_zbf[:].rearrange("l t b -> l (t b)"))
                    nc.sync.dma_start(
                        outs["qlv_out"].ap()[g].rearrange("l t b -> l (t b)"),
                        ob_qlv[:].rearrange("l t b -> l (t b)"))
                    drain(1)

            # tail: remaining prior blocks
            enqueue_prior_block(n_blocks - 2)
            enqueue_prior_block(n_blocks - 1)
            drain(len(pwork))

    nc.compile()
    return nc


# --------------------------------------------------------------------------
# Host-side data prep
# --------------------------------------------------------------------------

def prep_inputs(encoder_features, prev_latent, eps,
                pW1, pb1, pW2, pb2, pW3, pb3,
                qW1, qb1, qW2, qb2, qW3, qb3, n_steps=S):
    bf = ml_dtypes.bfloat16
    f32 = np.float32
    nco = NCORES

    enc = np.asarray(encoder_features, f32)[:, :n_steps]
    epsv = np.asarray(eps, f32)[:, :n_steps]
    prev = np.asarray(prev_latent, f32)

    # [core, kc, d, s, b]
    hT = np.ascontiguousarray(
        enc.reshape(nco, BC, n_steps, 2, 128).transpose(0, 3, 4, 2, 1)
    ).reshape(nco, 2, 128, n_steps * BC).astype(bf)
    epsT = np.ascontiguousarray(
        epsv.reshape(nco, BC, n_steps, 128).transpose(0, 3, 2, 1)
    ).reshape(nco, 128, n_steps * BC).astype(bf)
    z0T = np.ascontiguousarray(
        prev.reshape(nco, BC, 128).transpose(0, 2, 1)).astype(bf)

    def wchunks(wmat):  # [256, H'] -> [2, 128, H']
        return np.ascontiguousarray(np.asarray(wmat, f32).reshape(2, 128, -1)).astype(bf)

    qW1 = np.asarray(qW1, f32)
    pW1 = np.asarray(pW1, f32)
    shared = {
        "qW1h": wchunks(qW1[0:256] + qW1[384:640]),
        "qW1z": np.ascontiguousarray(qW1[256:384]).astype(bf),
        "qW2": wchunks(qW2), "qW3": wchunks(qW3),
        "pW1h": wchunks(pW1[0:256]),
        "pW1z": np.ascontiguousarray(pW1[256:384]).astype(bf),
        "pW2": wchunks(pW2), "pW3": wchunks(pW3),
        "B1q": np.ascontiguousarray(np.broadcast_to(
            np.asarray(qb1, f32).reshape(2, 128).T[:, :, None],
            (128, 2, BC))).reshape(128, 256).astype(bf),
        "B2q": np.ascontiguousarray(np.broadcast_to(
            np.asarray(qb2, f32).reshape(2, 128).T[:, :, None],
            (128, 2, BC))).reshape(128, 256).astype(bf),
        "ident": np.eye(128, dtype=f32).astype(bf),
        "qb3m": np.asarray(qb3, f32)[0:128].reshape(128, 1).copy(),
        "qb3lv": np.asarray(qb3, f32)[128:256].reshape(128, 1).copy(),
        "qb3lvh": (0.5 * np.asarray(qb3, f32)[128:256]).reshape(128, 1).copy(),
        "pb1c": np.ascontiguousarray(np.asarray(pb1, f32).reshape(2, 128).T),
        "pb2c": np.ascontiguousarray(np.asarray(pb2, f32).reshape(2, 128).T),
        "pb3m": np.asarray(pb3, f32)[0:128].reshape(128, 1).copy(),
        "pb3lv": np.asarray(pb3, f32)[128:256].reshape(128, 1).copy(),
    }
    in_maps = []
    for c in range(nco):
        m = {"hT": hT[c], "epsT": epsT[c], "z0T": z0T[c]}
        m.update(shared)
        in_maps.append(m)
    return in_maps


def unshard(results, n_steps=S):
    """results: per-core dicts of bf16 block tensors -> five [B,S,128] f32."""
    f32 = np.float32
    nb = n_steps // 4

    def blocks_to_bst(a):  # [nb, 128, 4, BC] -> [BC, nsteps, 128]
        return np.asarray(a, f32).transpose(3, 0, 2, 1).reshape(BC, n_steps, 128)

    def full(name, shift=False):
        per = []
        for r in results:
            a = np.asarray(r[name], f32)
            if shift:  # [nb+1, 128, 4, BC], slot k = val_{k-1}
                flat = a.transpose(3, 0, 2, 1).reshape(BC, (nb + 1) * 4, 128)
                per.append(flat[:, 1:n_steps + 1])
            else:
                per.append(blocks_to_bst(a))
        return np.ascontiguousarray(np.concatenate(per, axis=0))

    return (full("z_out"), full("pm_out"), full("plv_out"),
            full("qm_out", shift=True), full("qlv_out"))


_NC_CACHE = {}


def get_nc(n_steps=S):
    if n_steps not in _NC_CACHE:
        _NC_CACHE[n_steps] = build_nc(n_steps)
    return _NC_CACHE[n_steps]


def kernel(**inputs):
    in_maps = prep_inputs(**inputs)
    nc = get_nc(S)
    res = bass_utils.run_bass_kernel_spmd(
        nc, in_maps, core_ids=list(range(NCORES)), trace=False)
    return unshard(res.results)


# revision 28
# speedup vs baseline: 1.3537x; 1.0846x over previous
# Trainium2 Bass kernel for nn_ConditionalVariationalModule_75299366633595.
#
# Reference computation (see problem spec): a conditional VAE scan over
# S=256 timesteps. Per step t (batch B=1024):
#   prior_out = MLP3([h_t, z], pW*)          -> pm, plv      (2*128)
#   post_out  = MLP3([h_t, z, h_t], qW*)     -> qm, qlv
#   z_t = qm + eps_t * exp(0.5*qlv)
# Outputs: z, pm, plv, qm, qlv each [B, S, 128] (returned as a tuple).
#
# Strategy (8 cores, data-parallel over batch, 128 samples/core):
# - Feature-major on device ([feature, batch]); host does all transposes.
# - The posterior recurrence is latency-bound: per step the chain is
#   zmm -> relu1 -> L2 -> relu2 -> L3(lv) -> exp -> prod -> zmm'.
#   Chain ops are placed on the lowest-latency engines (DVE relus,
#   ACT exp, DVE 2x-bf16 multiply), the qlv half of L3 is computed
#   first so exp starts early, and everything else (prior MLP, h-part
#   matmuls, bias-ident matmuls, output staging) fills the PE bubbles.
# - z is never materialized on the critical path: layer-1 takes qm and
#   prod = eps*exp(0.5qlv+0.5b) as two rhs operands.
# - The prior MLP runs as batched N=512 GEMMs over 4-step token blocks,
#   interleaved at sub-step granularity, reading z and h from SBUF.
# - All outputs staged/DMA'd in bf16 (host upcasts); matmuls bf16.

import os
import numpy as np
import ml_dtypes

import concourse.bass as bass
import concourse.mybir as mybir
import concourse.tile as tile
from concourse import bacc
from concourse import bass_utils

AF = mybir.ActivationFunctionType
ALU = mybir.AluOpType
F32 = mybir.dt.float32
BF16 = mybir.dt.bfloat16

NCORES = 8
B_TOTAL = 1024
BC = B_TOTAL // NCORES  # 128 batch per core
S = 256
D = 256  # input dim
L = 128  # latent dim
H = 256  # hidden dim


# --------------------------------------------------------------------------
# Device program
# --------------------------------------------------------------------------

def build_nc(n_steps=S):
    """Build the per-core Bass program (SPMD across 8 cores)."""
    nc = bacc.Bacc("TRN2", target_bir_lowering=False, debug=False,
                   num_devices=NCORES)

    n_blocks = n_steps // 4
    assert n_steps % 4 == 0

    # ---- DRAM I/O ----
    hT = nc.dram_tensor("hT", [2, 128, n_steps * BC], BF16, kind="ExternalInput")
    epsT = nc.dram_tensor("epsT", [128, n_steps * BC], BF16, kind="ExternalInput")
    z0T = nc.dram_tensor("z0T", [128, BC], BF16, kind="ExternalInput")

    wspec = {
        "qW1h": [2, 128, 256], "qW1z": [128, 256],
        "qW2": [2, 128, 256], "qW3": [2, 128, 256],
        "pW1h": [2, 128, 256], "pW1z": [128, 256],
        "pW2": [2, 128, 256], "pW3": [2, 128, 256],
        "B1q": [128, 256], "B2q": [128, 256],
        "ident": [128, 128],
    }
    wdram = {k: nc.dram_tensor(k, shp, BF16, kind="ExternalInput")
             for k, shp in wspec.items()}
    bspec = {
        "qb3m": [128, 1], "qb3lv": [128, 1], "qb3lvh": [128, 1],
        "pb1c": [128, 2], "pb2c": [128, 2],
        "pb3m": [128, 1], "pb3lv": [128, 1],
    }
    bdram = {k: nc.dram_tensor(k, shp, F32, kind="ExternalInput")
             for k, shp in bspec.items()}

    # outputs, all bf16 (host upcasts); qm is slot-shifted by one step
    outs = {
        "z_out": nc.dram_tensor("z_out", [n_blocks, 128, 4, BC], BF16,
                                kind="ExternalOutput"),
        "qm_out": nc.dram_tensor("qm_out", [n_blocks + 1, 128, 4, BC], BF16,
                                 kind="ExternalOutput"),
        "qlv_out": nc.dram_tensor("qlv_out", [n_blocks, 128, 4, BC], BF16,
                                  kind="ExternalOutput"),
        "pm_out": nc.dram_tensor("pm_out", [n_blocks, 128, 4, BC], BF16,
                                 kind="ExternalOutput"),
        "plv_out": nc.dram_tensor("plv_out", [n_blocks, 128, 4, BC], BF16,
                                  kind="ExternalOutput"),
    }

    with tile.TileContext(nc) as tc:
        with (
            tc.tile_pool(name="const", bufs=1) as const,
            tc.tile_pool(name="hp", bufs=7) as hpool,
            tc.tile_pool(name="ep", bufs=5) as epool,
            tc.tile_pool(name="sp", bufs=3) as spool,
            tc.tile_pool(name="zb", bufs=6) as zpool,
            tc.tile_pool(name="blk", bufs=3) as bpool,
            tc.tile_pool(name="p3", bufs=3) as p3pool,
            tc.tile_pool(name="ps", bufs=2, space="PSUM") as psp,
        ):
            # ---- constants into SBUF; scan-critical weights first, and
            # spread across three DMA queues so loads overlap ----
            w = {}
            bias = {}

            def wload(k, eng):
                if len(wspec[k]) == 3:
                    t_ = const.tile([128, 2, 256], BF16, tag=k)
                    eng.dma_start(t_[:], wdram[k].ap().rearrange("k d h -> d k h"))
                elif k in ("B1q", "B2q"):  # [128, 2 chunks, 128 batch]
                    t_ = const.tile([128, 2, 128], BF16, tag=k)
                    eng.dma_start(t_[:], wdram[k].ap()
                                  .rearrange("d (c b) -> d c b", c=2))
                else:
                    t_ = const.tile(list(wspec[k]), BF16, tag=k)
                    eng.dma_start(t_[:], wdram[k].ap())
                w[k] = t_

            def bload(k, eng):
                t_ = const.tile(list(bspec[k]), F32, tag=k)
                eng.dma_start(t_[:], bdram[k].ap())
                bias[k] = t_

            # scan-critical constants first: ident/B1q on ACT (first matmul
            # needs them), z0/h/qW1 on SP, later-layer weights behind them
            for k in ("ident", "B1q"):
                wload(k, nc.scalar)
            z0t = const.tile([128, BC], BF16, tag="z0T")
            nc.sync.dma_start(z0t[:], z0T.ap())
            ident = w["ident"]

            # ---- streaming input tiles (4 steps per group) ----
            htiles, etiles = {}, {}

            def load_group(g):
                if g < 0 or g * 4 >= n_steps or g in htiles:
                    return
                ht = hpool.tile([128, 2, 4 * BC], BF16, tag="h")
                nc.sync.dma_start(
                    ht[:], hT.ap()[:, :, g * 4 * BC:(g + 1) * 4 * BC]
                    .rearrange("k d f -> d k f"))
                et = epool.tile([128, 4 * BC], BF16, tag="e")
                nc.sync.dma_start(et[:], epsT.ap()[:, g * 4 * BC:(g + 1) * 4 * BC])
                htiles[g] = ht
                etiles[g] = et

            load_group(0)
            for k in ("qW1h", "qW1z"):
                wload(k, nc.sync)
            for k in ("qW2", "B2q", "qW3"):
                wload(k, nc.scalar)
            for k in ("qb3m", "qb3lv", "qb3lvh"):
                bload(k, nc.scalar)
            load_group(1)
            load_group(2)

            # prior-side constants: on SP after the first input groups
            # (not needed until the first prior block drains, ~step 8)
            for k in ("pW1h", "pW1z", "pW2", "pW3"):
                wload(k, nc.sync)
            for k in ("pb1c", "pb2c", "pb3m", "pb3lv"):
                bload(k, nc.sync)

            zbftiles = {}   # block g -> ob_zbf tile (z_t bf16, slots 0..3)

            HC = [(0, slice(0, 128)), (1, slice(128, 256))]

            # ---------- prior MLP work queue (sub-step interleave) ----------
            # One ordered queue (emission order must respect data deps), but
            # units are engine-tagged: PE stall windows pull until they get a
            # matmul unit (ACT units encountered on the way are emitted too —
            # they are far behind their deps and execute immediately).
            from collections import deque
            pwork = deque()  # items: ("pe" | "act", closure)

            def drain_pe(n):
                done = 0
                while pwork and done < n:
                    kind, f = pwork.popleft()
                    f()
                    if kind == "pe":
                        done += 1

            def drain_act(n):
                done = 0
                while pwork and done < n and pwork[0][0] == "act":
                    pwork.popleft()[1]()
                    done += 1

            def enqueue_prior_block(j):
                """Prior MLP for tokens [4j, 4j+4) as a list of small work
                units; z and h come from SBUF (no DRAM round-trip)."""
                if j < 0 or j >= n_blocks:
                    return
                ht2 = htiles[j]
                zprev = z0t[:] if j == 0 else zbftiles[j - 1][:, 3, :]
                zcur = zbftiles[j]
                st = {}

                def mk_ps1(hc, hs):
                    def f():
                        ps = psp.tile([128, 4 * BC], F32, tag="pps",
                                      bufs=2, name=f"pps1_{j}_{hc}")
                        st[("ps1", hc)] = ps
                        nc.tensor.matmul(ps[:], w["pW1h"][:, 0, hs],
                                         ht2[:, 0, :], start=True, stop=False)
                        nc.tensor.matmul(ps[:], w["pW1h"][:, 1, hs],
                                         ht2[:, 1, :], start=False, stop=False)
                    return f

                def mk_ps1z(hc, hs):
                    def f():
                        ps = st[("ps1", hc)]
                        nc.tensor.matmul(ps[:, 0:BC], w["pW1z"][:, hs],
                                         zprev, start=False, stop=False)
                        nc.tensor.matmul(ps[:, BC:4 * BC], w["pW1z"][:, hs],
                                         zcur[:, 0:3, :], start=False, stop=True)
                    return f

                def mk_relu(layer, hc, bkey, half):
                    def f():
                        if layer not in st:
                            st[layer] = p3pool.tile([128, 2, 4 * BC], BF16,
                                                    tag=f"h{layer}",
                                                    name=f"ph{layer}_{j}")
                        hw = 2 * BC
                        hsl = slice(half * hw, (half + 1) * hw)
                        nc.scalar.activation(st[layer][:, hc, hsl],
                                             st[(f"ps{layer}", hc)][:, hsl],
                                             AF.Relu, bias=bias[bkey][:, hc:hc + 1])
                    return f

                def mk_ps(layer, hc, hs, wkey, prev):
                    def f():
                        ps = psp.tile([128, 4 * BC], F32, tag="pps",
                                      bufs=2, name=f"pps{layer}_{j}_{hc}")
                        st[(f"ps{layer}", hc)] = ps
                        for kc in (0, 1):
                            nc.tensor.matmul(ps[:], w[wkey][:, kc, hs],
                                             st[prev][:, kc, :],
                                             start=(kc == 0), stop=(kc == 1))
                    return f

                def mk_out(hc, bkey, oname, half):
                    def f():
                        key = f"o{oname}"
                        if key not in st:
                            st[key] = p3pool.tile([128, 4 * BC], BF16,
                                                  tag=key, name=f"{key}_{j}")
                        hw = 2 * BC
                        hsl = slice(half * hw, (half + 1) * hw)
                        nc.scalar.activation(st[key][:, hsl],
                                             st[("ps3", hc)][:, hsl],
                                             AF.Identity,
                                             bias=bias[bkey][:, 0:1])
                        if half == 1:
                            nc.sync.dma_start(
                                outs[oname].ap()[j].rearrange("l t b -> l (t b)"),
                                st[key][:])
                    return f

                for hc, hs in HC:
                    pwork.append(("pe", mk_ps1(hc, hs)))
                    pwork.append(("pe", mk_ps1z(hc, hs)))
                for hc in (0, 1):
                    for half in (0, 1):
                        pwork.append(("act", mk_relu(1, hc, "pb1c", half)))
                for hc, hs in HC:
                    pwork.append(("pe", mk_ps(2, hc, hs, "pW2", 1)))
                for hc in (0, 1):
                    for half in (0, 1):
                        pwork.append(("act", mk_relu(2, hc, "pb2c", half)))
                for hc, hs in HC:
                    pwork.append(("pe", mk_ps(3, hc, hs, "pW3", 2)))
                for half in (0, 1):
                    pwork.append(("act", mk_out(0, "pb3m", "pm_out", half)))
                for half in (0, 1):
                    pwork.append(("act", mk_out(1, "pb3lv", "plv_out", half)))

            # ================= the scan =================
            # Two batch half-groups (columns 0:64 / 64:128 of each step) run
            # as independent recurrence chains, phase-offset by half a step:
            # narrower chain ops (relu/exp/mul on 64-wide batch) shorten the
            # per-step dependency cycle, and each group's matmuls fill the
            # other group's stall windows. Emission interleaves the groups'
            # pipeline stages so every PE instruction's deps are ready when
            # it reaches the head of the in-order queue.
            GW = BC // 2  # 64: group width
            qm_prev = [None, None]
            prod_prev = [None, None]
            qmblks, prodblks, qlvtiles = {}, {}, {}
            psum1 = [None, None]
            psum3 = [None, None]
            h1 = [None, None]
            h2 = [None, None]

            def get_blk(d, bidx, tag):
                if bidx not in d:
                    d[bidx] = bpool.tile([128, 4, BC], BF16, tag=tag,
                                         name=f"{tag}_{bidx}")
                return d[bidx]

            def gsl(X, t):
                sl = t % 4
                return slice(sl * BC + X * GW, sl * BC + (X + 1) * GW)

            def emit_l1_base(X, t):
                """ident-bias + h-part of posterior layer 1, group X step t."""
                ht = htiles[t // 4]
                bs = gsl(X, t)
                ps = psp.tile([128, 2, GW], F32, tag=f"l1{X}", bufs=1,
                              name=f"psum1_{X}_{t}")
                nc.tensor.matmul(ps[:], ident[:], w["B1q"][:, :, X * GW:(X + 1) * GW],
                                 start=True, stop=False)
                for hc, hs in HC:
                    for kc in (0, 1):
                        nc.tensor.matmul(ps[:, hc, :], w["qW1h"][:, kc, hs],
                                         ht[:, kc, bs], start=False, stop=False)
                psum1[X] = ps

            def emit_l1z(X, t):
                ps = psum1[X]
                if t == 0:
                    for hc, hs in HC:
                        nc.tensor.matmul(ps[:, hc, :], w["qW1z"][:, hs],
                                         z0t[:, X * GW:(X + 1) * GW],
                                         start=False, stop=(hc == 1))
                else:
                    for hc, hs in HC:
                        nc.tensor.matmul(ps[:, hc, :], w["qW1z"][:, hs],
                                         prod_prev[X], start=False, stop=False)
                    for hc, hs in HC:
                        nc.tensor.matmul(ps[:, hc, :], w["qW1z"][:, hs],
                                         qm_prev[X], start=False,
                                         stop=(hc == 1))
                ht_ = spool.tile([128, 2, GW], BF16, tag=f"h1{X}",
                                 name=f"h1_{X}_{t}")
                nc.vector.tensor_scalar_max(ht_[:], ps[:], 0.0)
                h1[X] = ht_

            def emit_l2(X, t):
                ps = psp.tile([128, 2, GW], F32, tag=f"l23{X}", bufs=1,
                              name=f"psum2_{X}_{t}")
                nc.tensor.matmul(ps[:], ident[:], w["B2q"][:, :, X * GW:(X + 1) * GW],
                                 start=True, stop=False)
                for hc, hs in HC:
                    for kc in (0, 1):
                        nc.tensor.matmul(ps[:, hc, :], w["qW2"][:, kc, hs],
                                         h1[X][:, kc, :],
                                         start=False, stop=(hc == 1 and kc == 1))
                ht_ = spool.tile([128, 2, GW], BF16, tag=f"h2{X}",
                                 name=f"h2_{X}_{t}")
                nc.vector.tensor_scalar_max(ht_[:], ps[:], 0.0)
                h2[X] = ht_

            def emit_l3(X, t):
                # qlv half first in its own bank (it gates exp); the qm half
                # goes to a bank time-shared with layer 2
                pslv = psp.tile([128, GW], F32, tag=f"lv{X}", bufs=1,
                                name=f"psum3lv_{X}_{t}")
                for kc in (0, 1):
                    nc.tensor.matmul(pslv[:], w["qW3"][:, kc, 128:256],
                                     h2[X][:, kc, :],
                                     start=(kc == 0), stop=(kc == 1))
                psqm = psp.tile([128, GW], F32, tag=f"l23{X}", bufs=1,
                                name=f"psum3qm_{X}_{t}")
                for kc in (0, 1):
                    nc.tensor.matmul(psqm[:], w["qW3"][:, kc, 0:128],
                                     h2[X][:, kc, :],
                                     start=(kc == 0), stop=(kc == 1))
                psum3[X] = (psqm, pslv)

            def emit_tail(X, t):
                """exp/qm/prod/z/qlv for group X step t (ACT/DVE/Pool ops)."""
                et = etiles[t // 4]
                ws = (t + 1) % 4
                bidx = (t + 1) // 4
                psqm, pslv = psum3[X]
                eh = spool.tile([128, GW], BF16, tag=f"eh{X}",
                                name=f"ehalf_{X}_{t}")
                nc.scalar.activation(eh[:], pslv[:], AF.Exp,
                                     bias=bias["qb3lvh"][:, 0:1], scale=0.5)
                qm_prev[X] = get_blk(qmblks, bidx, "qmb")[:, ws,
                                                          X * GW:(X + 1) * GW]
                prod_prev[X] = get_blk(prodblks, bidx, "prb")[:, ws,
                                                              X * GW:(X + 1) * GW]
                sl = t % 4
                nc.vector.tensor_scalar_add(
                    qlvtiles[t // 4][:, sl, X * GW:(X + 1) * GW], pslv[:],
                    bias["qb3lv"][:, 0:1])
                nc.vector.tensor_scalar_add(qm_prev[X], psqm[:],
                                            bias["qb3m"][:, 0:1])
                nc.vector.tensor_tensor(prod_prev[X], eh[:], et[:, gsl(X, t)],
                                        ALU.mult)
                nc.gpsimd.tensor_add(zbftiles[t // 4][:, sl, X * GW:(X + 1) * GW],
                                     qm_prev[X], prod_prev[X])

            # prologue: L1 base for both groups at t=0
            emit_l1_base(0, 0)
            emit_l1_base(1, 0)

            for t in range(n_steps):
                g, sl = t // 4, t % 4
                ws = (t + 1) % 4
                if sl == 0:
                    load_group(g + 3)
                    enqueue_prior_block(g - 2)
                    if g == n_blocks - 1:
                        enqueue_prior_block(n_blocks - 2)
                if sl == 2 and g == n_blocks - 1:
                    enqueue_prior_block(n_blocks - 1)

                if sl == 0:
                    zbftiles[g] = zpool.tile([128, 4, BC], BF16, tag="zb",
                                             name=f"zbf_{g}")
                    qlvtiles[g] = spool.tile([128, 4, BC], BF16, tag="qlvb",
                                             bufs=2, name=f"qlv_{g}")

                # ---------- interleaved emission ----------
                emit_l1z(0, t)                       # A chain head
                if t > 0:
                    emit_l3(1, t - 1)                # B finishing step t-1
                    emit_tail(1, t - 1)
                    drain_act(1)
                emit_l1_base(1, t)                   # B's next-step L1 fill

                # previous-block output DMAs (all writes now emitted)
                if sl == 0 and g > 0:
                    nc.sync.dma_start(
                        outs["z_out"].ap()[g - 1].rearrange("l t b -> l (t b)"),
                        zbftiles[g - 1][:].rearrange("l t b -> l (t b)"))
                    nc.sync.dma_start(
                        outs["qlv_out"].ap()[g - 1].rearrange("l t b -> l (t b)"),
                        qlvtiles[g - 1][:].rearrange("l t b -> l (t b)"))
                if ws == 0 and t > 0:
                    nc.sync.dma_start(
                        outs["qm_out"].ap()[t // 4].rearrange("l t b -> l (t b)"),
                        qmblks[t // 4][:].rearrange("l t b -> l (t b)"))

                drain_pe(1)
                emit_l2(0, t)                        # A layer 2
                emit_l1z(1, t)                       # B chain head
                emit_l3(0, t)                        # A layer 3
                emit_tail(0, t)                      # A tail elementwise
                drain_act(1)
                if t + 1 < n_steps:
                    emit_l1_base(0, t + 1)           # A's next-step L1 fill
                drain_pe(1)
                emit_l2(1, t)                        # B layer 2

            # epilogue: B finishes the last step, then final DMAs
            emit_l3(1, n_steps - 1)
            emit_tail(1, n_steps - 1)
            nc.sync.dma_start(
                outs["z_out"].ap()[n_blocks - 1].rearrange("l t b -> l (t b)"),
                zbftiles[n_blocks - 1][:].rearrange("l t b -> l (t b)"))
            nc.sync.dma_start(
                outs["qlv_out"].ap()[n_blocks - 1].rearrange("l t b -> l (t b)"),
                qlvtiles[n_blocks - 1][:].rearrange("l t b -> l (t b)"))
            nc.sync.dma_start(
                outs["qm_out"].ap()[n_blocks].rearrange("l t b -> l (t b)"),
                qmblks[n_blocks][:].rearrange("l t b -> l (t b)"))

            # tail: whatever prior work remains
            drain_pe(len(pwork))

    nc.compile()
    return nc


# --------------------------------------------------------------------------
# Host-side data prep
# --------------------------------------------------------------------------

def prep_inputs(encoder_features, prev_latent, eps,
                pW1, pb1, pW2, pb2, pW3, pb3,
                qW1, qb1, qW2, qb2, qW3, qb3, n_steps=S):
    bf = ml_dtypes.bfloat16
    f32 = np.float32
    nco = NCORES

    enc = np.asarray(encoder_features, f32)[:, :n_steps]
    epsv = np.asarray(eps, f32)[:, :n_steps]
    prev = np.asarray(prev_latent, f32)

    # [core, kc, d, s, b]
    hT = np.ascontiguousarray(
        enc.reshape(nco, BC, n_steps, 2, 128).transpose(0, 3, 4, 2, 1)
    ).reshape(nco, 2, 128, n_steps * BC).astype(bf)
    epsT = np.ascontiguousarray(
        epsv.reshape(nco, BC, n_steps, 128).transpose(0, 3, 2, 1)
    ).reshape(nco, 128, n_steps * BC).astype(bf)
    z0T = np.ascontiguousarray(
        prev.reshape(nco, BC, 128).transpose(0, 2, 1)).astype(bf)

    def wchunks(wmat):  # [256, H'] -> [2, 128, H']
        return np.ascontiguousarray(np.asarray(wmat, f32).reshape(2, 128, -1)).astype(bf)

    qW1 = np.asarray(qW1, f32)
    pW1 = np.asarray(pW1, f32)
    shared = {
        "qW1h": wchunks(qW1[0:256] + qW1[384:640]),
        "qW1z": np.ascontiguousarray(qW1[256:384]).astype(bf),
        "qW2": wchunks(qW2), "qW3": wchunks(qW3),
        "pW1h": wchunks(pW1[0:256]),
        "pW1z": np.ascontiguousarray(pW1[256:384]).astype(bf),
        "pW2": wchunks(pW2), "pW3": wchunks(pW3),
        "B1q": np.ascontiguousarray(np.broadcast_to(
            np.asarray(qb1, f32).reshape(2, 128).T[:, :, None],
            (128, 2, BC))).reshape(128, 256).astype(bf),
        "B2q": np.ascontiguousarray(np.broadcast_to(
            np.asarray(qb2, f32).reshape(2, 128).T[:, :, None],
            (128, 2, BC))).reshape(128, 256).astype(bf),
        "ident": np.eye(128, dtype=f32).astype(bf),
        "qb3m": np.asarray(qb3, f32)[0:128].reshape(128, 1).copy(),
        "qb3lv": np.asarray(qb3, f32)[128:256].reshape(128, 1).copy(),
        "qb3lvh": (0.5 * np.asarray(qb3, f32)[128:256]).reshape(128, 1).copy(),
        "pb1c": np.ascontiguousarray(np.asarray(pb1, f32).reshape(2, 128).T),
        "pb2c": np.ascontiguousarray(np.asarray(pb2, f32).reshape(2, 128).T),
        "pb3m": np.asarray(pb3, f32)[0:128].reshape(128, 1).copy(),
        "pb3lv": np.asarray(pb3, f32)[128:256].reshape(128, 1).copy(),
    }
    in_maps = []
    for c in range(nco):
        m = {"hT": hT[c], "epsT": epsT[c], "z0T": z0T[c]}
        m.update(shared)
        in_maps.append(m)
    return in_maps


def unshard(results, n_steps=S):
    """results: per-core dicts of bf16 block tensors -> five [B,S,128] f32."""
    f32 = np.float32
    nb = n_steps // 4

    def blocks_to_bst(a):  # [nb, 128, 4, BC] -> [BC, nsteps, 128]
        return np.asarray(a, f32).transpose(3, 0, 2, 1).reshape(BC, n_steps, 128)

    def full(name, shift=False):
        per = []
        for r in results:
            a = np.asarray(r[name], f32)
            if shift:  # [nb+1, 128, 4, BC], slot k = val_{k-1}
                flat = a.transpose(3, 0, 2, 1).reshape(BC, (nb + 1) * 4, 128)
                per.append(flat[:, 1:n_steps + 1])
            else:
                per.append(blocks_to_bst(a))
        return np.ascontiguousarray(np.concatenate(per, axis=0))

    return (full("z_out"), full("pm_out"), full("plv_out"),
            full("qm_out", shift=True), full("qlv_out"))


_NC_CACHE = {}


def get_nc(n_steps=S):
    if n_steps not in _NC_CACHE:
        _NC_CACHE[n_steps] = build_nc(n_steps)
    return _NC_CACHE[n_steps]


def kernel(**inputs):
    in_maps = prep_inputs(**inputs)
    nc = get_nc(S)
    res = bass_utils.run_bass_kernel_spmd(
        nc, in_maps, core_ids=list(range(NCORES)), trace=False)
    return unshard(res.results)


# revision 30
# speedup vs baseline: 1.3844x; 1.0227x over previous
# Trainium2 Bass kernel for nn_ConditionalVariationalModule_75299366633595.
#
# Reference computation (see problem spec): a conditional VAE scan over
# S=256 timesteps. Per step t (batch B=1024):
#   prior_out = MLP3([h_t, z], pW*)          -> pm, plv      (2*128)
#   post_out  = MLP3([h_t, z, h_t], qW*)     -> qm, qlv
#   z_t = qm + eps_t * exp(0.5*qlv)
# Outputs: z, pm, plv, qm, qlv each [B, S, 128] (returned as a tuple).
#
# Strategy (8 cores, data-parallel over batch, 128 samples/core):
# - Feature-major on device ([feature, batch]); host does all transposes.
# - The posterior recurrence is latency-bound: per step the chain is
#   zmm -> relu1 -> L2 -> relu2 -> L3(lv) -> exp -> prod -> zmm'.
#   Chain ops are placed on the lowest-latency engines (DVE relus,
#   ACT exp, DVE 2x-bf16 multiply), the qlv half of L3 is computed
#   first so exp starts early, and everything else (prior MLP, h-part
#   matmuls, bias-ident matmuls, output staging) fills the PE bubbles.
# - z is never materialized on the critical path: layer-1 takes qm and
#   prod = eps*exp(0.5qlv+0.5b) as two rhs operands.
# - The prior MLP runs as batched N=512 GEMMs over 4-step token blocks,
#   interleaved at sub-step granularity, reading z and h from SBUF.
# - All outputs staged/DMA'd in bf16 (host upcasts); matmuls bf16.

import os
import numpy as np
import ml_dtypes

import concourse.bass as bass
import concourse.mybir as mybir
import concourse.tile as tile
from concourse import bacc
from concourse import bass_utils

AF = mybir.ActivationFunctionType
ALU = mybir.AluOpType
F32 = mybir.dt.float32
BF16 = mybir.dt.bfloat16

NCORES = 8
B_TOTAL = 1024
BC = B_TOTAL // NCORES  # 128 batch per core
S = 256
D = 256  # input dim
L = 128  # latent dim
H = 256  # hidden dim


# --------------------------------------------------------------------------
# Device program
# --------------------------------------------------------------------------

def build_nc(n_steps=S):
    """Build the per-core Bass program (SPMD across 8 cores)."""
    nc = bacc.Bacc("TRN2", target_bir_lowering=False, debug=False,
                   num_devices=NCORES)

    n_blocks = n_steps // 4
    assert n_steps % 4 == 0

    # ---- DRAM I/O ----
    hT = nc.dram_tensor("hT", [2, 128, n_steps * BC], BF16, kind="ExternalInput")
    epsT = nc.dram_tensor("epsT", [128, n_steps * BC], BF16, kind="ExternalInput")
    z0T = nc.dram_tensor("z0T", [128, BC], BF16, kind="ExternalInput")

    wspec = {
        "qW1h": [2, 128, 256], "qW1z": [128, 256],
        "qW2": [2, 128, 256], "qW3": [2, 128, 256],
        "pW1h": [2, 128, 256], "pW1z": [128, 256],
        "pW2": [2, 128, 256], "pW3": [2, 128, 256],
        "B1q": [128, 256], "B2q": [128, 256],
        "ident": [128, 128],
    }
    wdram = {k: nc.dram_tensor(k, shp, BF16, kind="ExternalInput")
             for k, shp in wspec.items()}
    bspec = {
        "qb3m": [128, 1], "qb3lv": [128, 1], "qb3lvh": [128, 1],
        "pb1c": [128, 2], "pb2c": [128, 2],
        "pb3m": [128, 1], "pb3lv": [128, 1],
    }
    bdram = {k: nc.dram_tensor(k, shp, F32, kind="ExternalInput")
             for k, shp in bspec.items()}

    # outputs, all bf16 (host upcasts); qm is slot-shifted by one step
    outs = {
        "z_out": nc.dram_tensor("z_out", [n_blocks, 128, 4, BC], BF16,
                                kind="ExternalOutput"),
        "qm_out": nc.dram_tensor("qm_out", [n_blocks + 1, 128, 4, BC], BF16,
                                 kind="ExternalOutput"),
        "qlv_out": nc.dram_tensor("qlv_out", [n_blocks, 128, 4, BC], BF16,
                                  kind="ExternalOutput"),
        "pm_out": nc.dram_tensor("pm_out", [n_blocks, 128, 4, BC], BF16,
                                 kind="ExternalOutput"),
        "plv_out": nc.dram_tensor("plv_out", [n_blocks, 128, 4, BC], BF16,
                                  kind="ExternalOutput"),
    }

    with tile.TileContext(nc) as tc:
        with (
            tc.tile_pool(name="const", bufs=1) as const,
            tc.tile_pool(name="hp", bufs=7) as hpool,
            tc.tile_pool(name="ep", bufs=5) as epool,
            tc.tile_pool(name="sp", bufs=3) as spool,
            tc.tile_pool(name="zb", bufs=6) as zpool,
            tc.tile_pool(name="blk", bufs=3) as bpool,
            tc.tile_pool(name="p3", bufs=3) as p3pool,
            tc.tile_pool(name="ps", bufs=2, space="PSUM") as psp,
        ):
            # ---- constants into SBUF; scan-critical weights first, and
            # spread across three DMA queues so loads overlap ----
            w = {}
            bias = {}

            def wload(k, eng):
                if len(wspec[k]) == 3:
                    t_ = const.tile([128, 2, 256], BF16, tag=k)
                    eng.dma_start(t_[:], wdram[k].ap().rearrange("k d h -> d k h"))
                elif k in ("B1q", "B2q"):  # [128, 2 chunks, 128 batch]
                    t_ = const.tile([128, 2, 128], BF16, tag=k)
                    eng.dma_start(t_[:], wdram[k].ap()
                                  .rearrange("d (c b) -> d c b", c=2))
                else:
                    t_ = const.tile(list(wspec[k]), BF16, tag=k)
                    eng.dma_start(t_[:], wdram[k].ap())
                w[k] = t_

            def bload(k, eng):
                t_ = const.tile(list(bspec[k]), F32, tag=k)
                eng.dma_start(t_[:], bdram[k].ap())
                bias[k] = t_

            # scan-critical constants first: ident/B1q on ACT (first matmul
            # needs them), z0/h/qW1 on SP, later-layer weights behind them
            for k in ("ident", "B1q"):
                wload(k, nc.scalar)
            z0t = const.tile([128, BC], BF16, tag="z0T")
            nc.sync.dma_start(z0t[:], z0T.ap())
            ident = w["ident"]

            # ---- streaming input tiles (4 steps per group) ----
            htiles, etiles = {}, {}

            def load_group(g):
                if g < 0 or g * 4 >= n_steps or g in htiles:
                    return
                ht = hpool.tile([128, 2, 4 * BC], BF16, tag="h")
                nc.sync.dma_start(
                    ht[:], hT.ap()[:, :, g * 4 * BC:(g + 1) * 4 * BC]
                    .rearrange("k d f -> d k f"))
                et = epool.tile([128, 4 * BC], BF16, tag="e")
                nc.sync.dma_start(et[:], epsT.ap()[:, g * 4 * BC:(g + 1) * 4 * BC])
                htiles[g] = ht
                etiles[g] = et

            load_group(0)
            for k in ("qW1h", "qW1z"):
                wload(k, nc.sync)
            for k in ("qW2", "B2q", "qW3"):
                wload(k, nc.scalar)
            for k in ("qb3m", "qb3lv", "qb3lvh"):
                bload(k, nc.scalar)
            load_group(1)
            load_group(2)

            # prior-side constants: on SP after the first input groups
            # (not needed until the first prior block drains, ~step 8)
            for k in ("pW1h", "pW1z", "pW2", "pW3"):
                wload(k, nc.sync)
            for k in ("pb1c", "pb2c", "pb3m", "pb3lv"):
                bload(k, nc.sync)

            zbftiles = {}   # block g -> ob_zbf tile (z_t bf16, slots 0..3)

            HC = [(0, slice(0, 128)), (1, slice(128, 256))]

            # ---------- prior MLP work queue (sub-step interleave) ----------
            # One ordered queue (emission order must respect data deps), but
            # units are engine-tagged: PE stall windows pull until they get a
            # matmul unit (ACT units encountered on the way are emitted too —
            # they are far behind their deps and execute immediately).
            from collections import deque
            pwork = deque()  # items: ("pe" | "act", closure)

            def drain_pe(n):
                done = 0
                while pwork and done < n:
                    kind, f = pwork.popleft()
                    f()
                    if kind == "pe":
                        done += 1

            def drain_act(n):
                done = 0
                while pwork and done < n and pwork[0][0] == "act":
                    pwork.popleft()[1]()
                    done += 1

            def enqueue_prior_block(j):
                """Prior MLP for tokens [4j, 4j+4) as a list of small work
                units; z and h come from SBUF (no DRAM round-trip)."""
                if j < 0 or j >= n_blocks:
                    return
                ht2 = htiles[j]
                zprev = z0t[:] if j == 0 else zbftiles[j - 1][:, 3, :]
                zcur = zbftiles[j]
                st = {}

                def mk_ps1(hc, hs):
                    def f():
                        ps = psp.tile([128, 4 * BC], F32, tag="pps",
                                      bufs=2, name=f"pps1_{j}_{hc}")
                        st[("ps1", hc)] = ps
                        nc.tensor.matmul(ps[:], w["pW1h"][:, 0, hs],
                                         ht2[:, 0, :], start=True, stop=False)
                        nc.tensor.matmul(ps[:], w["pW1h"][:, 1, hs],
                                         ht2[:, 1, :], start=False, stop=False)
                    return f

                def mk_ps1z(hc, hs):
                    def f():
                        ps = st[("ps1", hc)]
                        nc.tensor.matmul(ps[:, 0:BC], w["pW1z"][:, hs],
                                         zprev, start=False, stop=False)
                        nc.tensor.matmul(ps[:, BC:4 * BC], w["pW1z"][:, hs],
                                         zcur[:, 0:3, :], start=False, stop=True)
                    return f

                def mk_relu(layer, hc, bkey, half):
                    def f():
                        if layer not in st:
                            st[layer] = p3pool.tile([128, 2, 4 * BC], BF16,
                                                    tag=f"h{layer}",
                                                    name=f"ph{layer}_{j}")
                        hw = 2 * BC
                        hsl = slice(half * hw, (half + 1) * hw)
                        nc.scalar.activation(st[layer][:, hc, hsl],
                                             st[(f"ps{layer}", hc)][:, hsl],
                                             AF.Relu, bias=bias[bkey][:, hc:hc + 1])
                    return f

                def mk_ps(layer, hc, hs, wkey, prev):
                    def f():
                        ps = psp.tile([128, 4 * BC], F32, tag="pps",
                                      bufs=2, name=f"pps{layer}_{j}_{hc}")
                        st[(f"ps{layer}", hc)] = ps
                        for kc in (0, 1):
                            nc.tensor.matmul(ps[:], w[wkey][:, kc, hs],
                                             st[prev][:, kc, :],
                                             start=(kc == 0), stop=(kc == 1))
                    return f

                def mk_out(hc, bkey, oname, half):
                    def f():
                        key = f"o{oname}"
                        if key not in st:
                            st[key] = p3pool.tile([128, 4 * BC], BF16,
                                                  tag=key, name=f"{key}_{j}")
                        hw = 2 * BC
                        hsl = slice(half * hw, (half + 1) * hw)
                        nc.scalar.activation(st[key][:, hsl],
                                             st[("ps3", hc)][:, hsl],
                                             AF.Identity,
                                             bias=bias[bkey][:, 0:1])
                        if half == 1:
                            nc.sync.dma_start(
                                outs[oname].ap()[j].rearrange("l t b -> l (t b)"),
                                st[key][:])
                    return f

                for hc, hs in HC:
                    pwork.append(("pe", mk_ps1(hc, hs)))
                    pwork.append(("pe", mk_ps1z(hc, hs)))
                for hc in (0, 1):
                    for half in (0, 1):
                        pwork.append(("act", mk_relu(1, hc, "pb1c", half)))
                for hc, hs in HC:
                    pwork.append(("pe", mk_ps(2, hc, hs, "pW2", 1)))
                for hc in (0, 1):
                    for half in (0, 1):
                        pwork.append(("act", mk_relu(2, hc, "pb2c", half)))
                for hc, hs in HC:
                    pwork.append(("pe", mk_ps(3, hc, hs, "pW3", 2)))
                for half in (0, 1):
                    pwork.append(("act", mk_out(0, "pb3m", "pm_out", half)))
                for half in (0, 1):
                    pwork.append(("act", mk_out(1, "pb3lv", "plv_out", half)))

            # ================= the scan =================
            # Two batch half-groups (columns 0:64 / 64:128 of each step) run
            # as independent recurrence chains, phase-offset by half a step:
            # narrower chain ops (relu/exp/mul on 64-wide batch) shorten the
            # per-step dependency cycle, and each group's matmuls fill the
            # other group's stall windows. Emission interleaves the groups'
            # pipeline stages so every PE instruction's deps are ready when
            # it reaches the head of the in-order queue.
            GW = BC // 2  # 64: group width
            qm_prev = [None, None]
            prod_prev = [None, None]
            qmblks, prodblks, qlvtiles = {}, {}, {}
            psum1 = [None, None]
            psum3 = [None, None]
            h1 = [None, None]
            h2 = [None, None]

            def get_blk(d, bidx, tag):
                if bidx not in d:
                    d[bidx] = bpool.tile([128, 4, BC], BF16, tag=tag,
                                         name=f"{tag}_{bidx}")
                return d[bidx]

            def gsl(X, t):
                sl = t % 4
                return slice(sl * BC + X * GW, sl * BC + (X + 1) * GW)

            def emit_l1_base(X, t):
                """ident-bias + h-part of posterior layer 1, group X step t."""
                ht = htiles[t // 4]
                bs = gsl(X, t)
                ps = psp.tile([128, 2, GW], F32, tag=f"l1{X}", bufs=1,
                              name=f"psum1_{X}_{t}")
                nc.tensor.matmul(ps[:], ident[:], w["B1q"][:, :, X * GW:(X + 1) * GW],
                                 start=True, stop=False)
                for hc, hs in HC:
                    for kc in (0, 1):
                        nc.tensor.matmul(ps[:, hc, :], w["qW1h"][:, kc, hs],
                                         ht[:, kc, bs], start=False, stop=False)
                psum1[X] = ps

            def emit_l1z(X, t):
                ps = psum1[X]
                if t == 0:
                    for hc, hs in HC:
                        nc.tensor.matmul(ps[:, hc, :], w["qW1z"][:, hs],
                                         z0t[:, X * GW:(X + 1) * GW],
                                         start=False, stop=(hc == 1))
                else:
                    for hc, hs in HC:
                        nc.tensor.matmul(ps[:, hc, :], w["qW1z"][:, hs],
                                         prod_prev[X], start=False, stop=False)
                    for hc, hs in HC:
                        nc.tensor.matmul(ps[:, hc, :], w["qW1z"][:, hs],
                                         qm_prev[X], start=False,
                                         stop=(hc == 1))
                ht_ = spool.tile([128, 2, GW], BF16, tag=f"h1{X}",
                                 name=f"h1_{X}_{t}")
                nc.vector.tensor_scalar_max(ht_[:], ps[:], 0.0)
                h1[X] = ht_

            def emit_l2(X, t):
                ps = psp.tile([128, 2, GW], F32, tag=f"l23{X}", bufs=1,
                              name=f"psum2_{X}_{t}")
                nc.tensor.matmul(ps[:], ident[:], w["B2q"][:, :, X * GW:(X + 1) * GW],
                                 start=True, stop=False)
                for hc, hs in HC:
                    for kc in (0, 1):
                        nc.tensor.matmul(ps[:, hc, :], w["qW2"][:, kc, hs],
                                         h1[X][:, kc, :],
                                         start=False, stop=(hc == 1 and kc == 1))
                ht_ = spool.tile([128, 2, GW], BF16, tag=f"h2{X}",
                                 name=f"h2_{X}_{t}")
                nc.vector.tensor_scalar_max(ht_[:], ps[:], 0.0)
                h2[X] = ht_

            def emit_l3(X, t):
                # qlv half first in its own bank (it gates exp); the qm half
                # goes to a bank time-shared with layer 2
                pslv = psp.tile([128, GW], F32, tag=f"lv{X}", bufs=1,
                                name=f"psum3lv_{X}_{t}")
                for kc in (0, 1):
                    nc.tensor.matmul(pslv[:], w["qW3"][:, kc, 128:256],
                                     h2[X][:, kc, :],
                                     start=(kc == 0), stop=(kc == 1))
                psqm = psp.tile([128, GW], F32, tag=f"l23{X}", bufs=1,
                                name=f"psum3qm_{X}_{t}")
                for kc in (0, 1):
                    nc.tensor.matmul(psqm[:], w["qW3"][:, kc, 0:128],
                                     h2[X][:, kc, :],
                                     start=(kc == 0), stop=(kc == 1))
                psum3[X] = (psqm, pslv)

            def emit_tail(X, t):
                """exp/qm/prod/z/qlv for group X step t (ACT/DVE/Pool ops)."""
                et = etiles[t // 4]
                ws = (t + 1) % 4
                bidx = (t + 1) // 4
                psqm, pslv = psum3[X]
                eh = spool.tile([128, GW], BF16, tag=f"eh{X}",
                                name=f"ehalf_{X}_{t}")
                nc.scalar.activation(eh[:], pslv[:], AF.Exp,
                                     bias=bias["qb3lvh"][:, 0:1], scale=0.5)
                qm_prev[X] = get_blk(qmblks, bidx, "qmb")[:, ws,
                                                          X * GW:(X + 1) * GW]
                prod_prev[X] = get_blk(prodblks, bidx, "prb")[:, ws,
                                                              X * GW:(X + 1) * GW]
                sl = t % 4
                nc.vector.tensor_scalar_add(qm_prev[X], psqm[:],
                                            bias["qb3m"][:, 0:1])
                nc.vector.tensor_tensor(prod_prev[X], eh[:], et[:, gsl(X, t)],
                                        ALU.mult)
                nc.scalar.activation(
                    qlvtiles[t // 4][:, sl, X * GW:(X + 1) * GW], pslv[:],
                    AF.Identity, bias=bias["qb3lv"][:, 0:1])
                nc.gpsimd.tensor_add(zbftiles[t // 4][:, sl, X * GW:(X + 1) * GW],
                                     qm_prev[X], prod_prev[X])

            # prologue: L1 base for both groups at t=0
            emit_l1_base(0, 0)
            emit_l1_base(1, 0)

            for t in range(n_steps):
                g, sl = t // 4, t % 4
                ws = (t + 1) % 4
                if sl == 0:
                    load_group(g + 3)
                    enqueue_prior_block(g - 2)
                    if g == n_blocks - 1:
                        enqueue_prior_block(n_blocks - 2)
                if sl == 2 and g == n_blocks - 1:
                    enqueue_prior_block(n_blocks - 1)

                if sl == 0:
                    zbftiles[g] = zpool.tile([128, 4, BC], BF16, tag="zb",
                                             name=f"zbf_{g}")
                    qlvtiles[g] = spool.tile([128, 4, BC], BF16, tag="qlvb",
                                             bufs=2, name=f"qlv_{g}")

                # ---------- interleaved emission ----------
                emit_l1z(0, t)                       # A chain head
                if t > 0:
                    emit_l3(1, t - 1)                # B finishing step t-1
                    emit_tail(1, t - 1)
                    drain_act(1)
                emit_l1_base(1, t)                   # B's next-step L1 fill

                # previous-block output DMAs (all writes now emitted)
                if sl == 0 and g > 0:
                    nc.sync.dma_start(
                        outs["z_out"].ap()[g - 1].rearrange("l t b -> l (t b)"),
                        zbftiles[g - 1][:].rearrange("l t b -> l (t b)"))
                    nc.sync.dma_start(
                        outs["qlv_out"].ap()[g - 1].rearrange("l t b -> l (t b)"),
                        qlvtiles[g - 1][:].rearrange("l t b -> l (t b)"))
                if ws == 0 and t > 0:
                    nc.sync.dma_start(
                        outs["qm_out"].ap()[t // 4].rearrange("l t b -> l (t b)"),
                        qmblks[t // 4][:].rearrange("l t b -> l (t b)"))

                drain_pe(1)
                emit_l2(0, t)                        # A layer 2
                emit_l1z(1, t)                       # B chain head
                emit_l3(0, t)                        # A layer 3
                emit_tail(0, t)                      # A tail elementwise
                drain_act(1)
                if t + 1 < n_steps:
                    emit_l1_base(0, t + 1)           # A's next-step L1 fill
                drain_pe(1)
                emit_l2(1, t)                        # B layer 2

            # epilogue: B finishes the last step, then final DMAs
            emit_l3(1, n_steps - 1)
            emit_tail(1, n_steps - 1)
            nc.sync.dma_start(
                outs["z_out"].ap()[n_blocks - 1].rearrange("l t b -> l (t b)"),
                zbftiles[n_blocks - 1][:].rearrange("l t b -> l (t b)"))
            nc.sync.dma_start(
                outs["qlv_out"].ap()[n_blocks - 1].rearrange("l t b -> l (t b)"),
                qlvtiles[n_blocks - 1][:].rearrange("l t b -> l (t b)"))
            nc.sync.dma_start(
                outs["qm_out"].ap()[n_blocks].rearrange("l t b -> l (t b)"),
                qmblks[n_blocks][:].rearrange("l t b -> l (t b)"))

            # tail: whatever prior work remains
            drain_pe(len(pwork))

    nc.compile()
    return nc


# --------------------------------------------------------------------------
# Host-side data prep
# --------------------------------------------------------------------------

def prep_inputs(encoder_features, prev_latent, eps,
                pW1, pb1, pW2, pb2, pW3, pb3,
                qW1, qb1, qW2, qb2, qW3, qb3, n_steps=S):
    bf = ml_dtypes.bfloat16
    f32 = np.float32
    nco = NCORES

    enc = np.asarray(encoder_features, f32)[:, :n_steps]
    epsv = np.asarray(eps, f32)[:, :n_steps]
    prev = np.asarray(prev_latent, f32)

    # [core, kc, d, s, b]
    hT = np.ascontiguousarray(
        enc.reshape(nco, BC, n_steps, 2, 128).transpose(0, 3, 4, 2, 1)
    ).reshape(nco, 2, 128, n_steps * BC).astype(bf)
    epsT = np.ascontiguousarray(
        epsv.reshape(nco, BC, n_steps, 128).transpose(0, 3, 2, 1)
    ).reshape(nco, 128, n_steps * BC).astype(bf)
    z0T = np.ascontiguousarray(
        prev.reshape(nco, BC, 128).transpose(0, 2, 1)).astype(bf)

    def wchunks(wmat):  # [256, H'] -> [2, 128, H']
        return np.ascontiguousarray(np.asarray(wmat, f32).reshape(2, 128, -1)).astype(bf)

    qW1 = np.asarray(qW1, f32)
    pW1 = np.asarray(pW1, f32)
    shared = {
        "qW1h": wchunks(qW1[0:256] + qW1[384:640]),
        "qW1z": np.ascontiguousarray(qW1[256:384]).astype(bf),
        "qW2": wchunks(qW2), "qW3": wchunks(qW3),
        "pW1h": wchunks(pW1[0:256]),
        "pW1z": np.ascontiguousarray(pW1[256:384]).astype(bf),
        "pW2": wchunks(pW2), "pW3": wchunks(pW3),
        "B1q": np.ascontiguousarray(np.broadcast_to(
            np.asarray(qb1, f32).reshape(2, 128).T[:, :, None],
            (128, 2, BC))).reshape(128, 256).astype(bf),
        "B2q": np.ascontiguousarray(np.broadcast_to(
            np.asarray(qb2, f32).reshape(2, 128).T[:, :, None],
            (128, 2, BC))).reshape(128, 256).astype(bf),
        "ident": np.eye(128, dtype=f32).astype(bf),
        "qb3m": np.asarray(qb3, f32)[0:128].reshape(128, 1).copy(),
        "qb3lv": np.asarray(qb3, f32)[128:256].reshape(128, 1).copy(),
        "qb3lvh": (0.5 * np.asarray(qb3, f32)[128:256]).reshape(128, 1).copy(),
        "pb1c": np.ascontiguousarray(np.asarray(pb1, f32).reshape(2, 128).T),
        "pb2c": np.ascontiguousarray(np.asarray(pb2, f32).reshape(2, 128).T),
        "pb3m": np.asarray(pb3, f32)[0:128].reshape(128, 1).copy(),
        "pb3lv": np.asarray(pb3, f32)[128:256].reshape(128, 1).copy(),
    }
    in_maps = []
    for c in range(nco):
        m = {"hT": hT[c], "epsT": epsT[c], "z0T": z0T[c]}
        m.update(shared)
        in_maps.append(m)
    return in_maps


def unshard(results, n_steps=S):
    """results: per-core dicts of bf16 block tensors -> five [B,S,128] f32."""
    f32 = np.float32
    nb = n_steps // 4

    def blocks_to_bst(a):  # [nb, 128, 4, BC] -> [BC, nsteps, 128]
        return np.asarray(a, f32).transpose(3, 0, 2, 1).reshape(BC, n_steps, 128)

    def full(name, shift=False):
        per = []
        for r in results:
            a = np.asarray(r[name], f32)
            if shift:  # [nb+1, 128, 4, BC], slot k = val_{k-1}
                flat = a.transpose(3, 0, 2, 1).reshape(BC, (nb + 1) * 4, 128)
                per.append(flat[:, 1:n_steps + 1])
            else:
                per.append(blocks_to_bst(a))
        return np.ascontiguousarray(np.concatenate(per, axis=0))

    return (full("z_out"), full("pm_out"), full("plv_out"),
            full("qm_out", shift=True), full("qlv_out"))


_NC_CACHE = {}


def get_nc(n_steps=S):
    if n_steps not in _NC_CACHE:
        _NC_CACHE[n_steps] = build_nc(n_steps)
    return _NC_CACHE[n_steps]


def kernel(**inputs):
    in_maps = prep_inputs(**inputs)
    nc = get_nc(S)
    res = bass_utils.run_bass_kernel_spmd(
        nc, in_maps, core_ids=list(range(NCORES)), trace=False)
    return unshard(res.results)
